# revision 1
# baseline (speedup 1.0000x reference)
"""Trainium2 Bass kernel for nn_CrossPixContrastiveL2.

Per sample (one per NeuronCore, N=8 samples / 8 cores):
  dist[p,q] = ||r_p||^2 + ||i_q||^2 - 2 r_p.i_q          (HW x HW, C=128)
  logit = exp(exp(-dist)/10)
  row[p] = sum_q logit*mask / (sum_q logit + eps)
  col[q] = sum_p logit*mask / (sum_p logit + eps)
  loss = masked mean of -log over foreground/nonzero entries

Device strategy per core:
  - Gram matmuls (K=C=128, fp32) into PSUM, with a K=2 "broadcast" matmul
    folding -(||r||^2 + ||i||^2)/2 into the same accumulation so that
    PSUM = -dist/2 directly.
  - ACT pass 1: e1 = Exp(2*PSUM)            (bf16 out)
  - ACT pass 2: logit = Exp(e1/10), fused accum_out -> row sums of logit
  - DVE scalar_tensor_tensor: (im_bcast == rm[p]) * logit with fused
    accum_out -> masked row sums (single op: mask+mul+reduce)
  - Column sums via label-onehot matmuls: lhsT = [onehot(rm) | ones]
    (128 x 22) against logit accumulated over row tiles -> per-label
    column masses T[l,q]; col_lm[q] = T[im[q],q] via an elementwise
    select + K=22 matmul.
Host does the tiny final -log / masked mean over 4x1024 values per core.
"""

import os
from contextlib import ExitStack

import numpy as np
import ml_dtypes

import concourse.bacc as bacc
import concourse.tile as tile
import concourse.mybir as mybir
from concourse.bass_utils import run_bass_kernel_spmd

N, C, H, W = 8, 128, 32, 32
HW = H * W
NCORES = 8
NK = HW // 128          # 8 row tiles of 128 pixels
L = 21                  # label values 0..20
LL = L + 1              # onehot columns + ones column
TEMPERATURE = 10.0
EPS = 1e-6

_BF16 = ml_dtypes.bfloat16

_PROGRAM = None


def _build_program():
    f32 = mybir.dt.float32
    bf16 = mybir.dt.bfloat16
    AF = mybir.ActivationFunctionType
    ALU = mybir.AluOpType

    nc = bacc.Bacc("TRN2", target_bir_lowering=False, debug=False,
                   num_devices=NCORES)

    rgb = nc.dram_tensor("rgb", (C, HW), f32, kind="ExternalInput").ap()
    irr = nc.dram_tensor("irr", (C, HW), f32, kind="ExternalInput").ap()
    # row0 = ones, row1 = -||r_p||^2/2  (per pixel p)
    nrh = nc.dram_tensor("nrh", (2, HW), f32, kind="ExternalInput").ap()
    # row0 = -||i_q||^2/2, row1 = ones
    nih = nc.dram_tensor("nih", (2, HW), f32, kind="ExternalInput").ap()
    # ir labels broadcast across partitions (bf16, exact for ints 0..20)
    imb = nc.dram_tensor("imb", (128, HW), bf16, kind="ExternalInput").ap()
    # [p, LL*k + l] = (rm[128k+p] == l) for l<21 ; 1.0 at l=21
    oh = nc.dram_tensor("oh", (128, NK * LL), bf16, kind="ExternalInput").ap()
    # rows 0..20 = onehot of im along q; row 21 = ones
    ohim = nc.dram_tensor("ohim", (LL, HW), f32, kind="ExternalInput").ap()
    # rm labels, transposed layout: [p, k] = rm[128k+p]
    rmf = nc.dram_tensor("rmf", (128, NK), f32, kind="ExternalInput").ap()
    # selector (22,2): col0 = 21 ones then 0 ; col1 = zeros then 1
    sel = nc.dram_tensor("sel", (LL, 2), f32, kind="ExternalInput").ap()

    # outputs: rows[:, 0:NK] = masked row sums, rows[:, NK:2NK] = row sums
    rows = nc.dram_tensor("rows", (128, 2 * NK), f32,
                          kind="ExternalOutput").ap()
    # cols[0] = masked col sums, cols[1] = col sums
    cols = nc.dram_tensor("cols", (2, HW), f32, kind="ExternalOutput").ap()

    with tile.TileContext(nc) as tc, ExitStack() as ctx:
        sb = ctx.enter_context(tc.tile_pool(name="sb", bufs=1))
        work = ctx.enter_context(tc.tile_pool(name="work", bufs=3))
        ps = ctx.enter_context(tc.tile_pool(name="ps", bufs=2, space="PSUM"))
        acc = ctx.enter_context(tc.tile_pool(name="acc", bufs=1, space="PSUM"))
        cps_pool = ctx.enter_context(
            tc.tile_pool(name="cps", bufs=1, space="PSUM"))

        rgb_s = sb.tile([C, HW], f32)
        nc.sync.dma_start(rgb_s[:], rgb)
        ir_s = sb.tile([C, HW], f32)
        nc.sync.dma_start(ir_s[:], irr)
        nrh_s = sb.tile([2, HW], f32)
        nc.sync.dma_start(nrh_s[:], nrh)
        nih_s = sb.tile([2, HW], f32)
        nc.sync.dma_start(nih_s[:], nih)
        imb_s = sb.tile([128, HW], bf16)
        nc.sync.dma_start(imb_s[:], imb)
        oh_s = sb.tile([128, NK * LL], bf16)
        nc.sync.dma_start(oh_s[:], oh)
        ohim_s = sb.tile([LL, HW], f32)
        nc.sync.dma_start(ohim_s[:], ohim)
        rmf_s = sb.tile([128, NK], f32)
        nc.sync.dma_start(rmf_s[:], rmf)
        sel_s = sb.tile([LL, 2], f32)
        nc.sync.dma_start(sel_s[:], sel)

        rows_s = sb.tile([128, 2 * NK], f32)
        # per-label column masses, accumulated across the NK row tiles
        TT = acc.tile([LL, HW], f32)

        for k in range(NK):
            G = ps.tile([128, HW], f32)
            for qh in range(2):
                q = qh * 512
                # PSUM <- -(nr+ni)/2 broadcast (K=2), then += r.i (K=128)
                nc.tensor.matmul(G[:, q:q + 512],
                                 nrh_s[:, k * 128:(k + 1) * 128],
                                 nih_s[:, q:q + 512],
                                 start=True, stop=False)
                nc.tensor.matmul(G[:, q:q + 512],
                                 rgb_s[:, k * 128:(k + 1) * 128],
                                 ir_s[:, q:q + 512],
                                 start=False, stop=True)
            e1 = work.tile([128, HW], bf16, tag="e1")
            nc.scalar.activation(e1[:], G[:], AF.Exp, scale=2.0)
            logit = work.tile([128, HW], bf16, tag="logit")
            nc.scalar.activation(logit[:], e1[:], AF.Exp,
                                 scale=1.0 / TEMPERATURE,
                                 accum_out=rows_s[:, NK + k:NK + k + 1])
            lm = work.tile([128, HW], bf16, tag="lm")
            nc.vector.scalar_tensor_tensor(
                lm[:], imb_s[:], rmf_s[:, k:k + 1], logit[:],
                op0=ALU.is_equal, op1=ALU.mult,
                accum_out=rows_s[:, k:k + 1])
            for qh in range(2):
                q = qh * 512
                nc.tensor.matmul(TT[:, q:q + 512],
                                 oh_s[:, LL * k:LL * (k + 1)],
                                 logit[:, q:q + 512],
                                 start=(k == 0), stop=(k == NK - 1))

        # col_lm[q] = TT[im[q], q]; col_logit[q] = TT[21, q]
        TM = sb.tile([LL, HW], f32)
        nc.vector.tensor_tensor(TM[:], TT[:], ohim_s[:], op=ALU.mult)
        cps = cps_pool.tile([2, HW], f32)
        for qh in range(2):
            q = qh * 512
            nc.tensor.matmul(cps[:, q:q + 512], sel_s[:], TM[:, q:q + 512],
                             start=True, stop=True)
        cols_s = sb.tile([2, HW], f32)
        nc.vector.tensor_copy(cols_s[:], cps[:])

        nc.sync.dma_start(rows, rows_s[:])
        nc.sync.dma_start(cols, cols_s[:])

    nc.compile()
    return nc


def _get_program():
    global _PROGRAM
    if _PROGRAM is None:
        _PROGRAM = _build_program()
    return _PROGRAM


def _make_in_map(rgb_map, ir_map, rgb_mask, ir_mask, n):
    f32 = np.float32
    rgb = np.ascontiguousarray(rgb_map[n].reshape(C, HW), dtype=f32)
    irr = np.ascontiguousarray(ir_map[n].reshape(C, HW), dtype=f32)
    rm = rgb_mask[n].reshape(HW)
    im = ir_mask[n].reshape(HW)

    nr = (rgb * rgb).sum(axis=0, dtype=f32)
    ni = (irr * irr).sum(axis=0, dtype=f32)
    ones = np.ones(HW, dtype=f32)
    nrh = np.stack([ones, -0.5 * nr]).astype(f32)
    nih = np.stack([-0.5 * ni, ones]).astype(f32)

    imb = np.broadcast_to(im.astype(_BF16), (128, HW)).copy()

    rmT = rm.reshape(NK, 128).T  # [p, k]
    oh = np.zeros((128, NK, LL), dtype=_BF16)
    oh[:, :, :L] = (rmT[:, :, None] == np.arange(L)[None, None, :])
    oh[:, :, L] = 1
    oh = oh.reshape(128, NK * LL)

    ohim = np.zeros((LL, HW), dtype=f32)
    ohim[:L] = (np.arange(L)[:, None] == im[None, :])
    ohim[L] = 1.0

    rmf = np.ascontiguousarray(rmT, dtype=f32)

    sel = np.zeros((LL, 2), dtype=f32)
    sel[:L, 0] = 1.0
    sel[L, 1] = 1.0

    return {"rgb": rgb, "irr": irr, "nrh": nrh, "nih": nih, "imb": imb,
            "oh": oh, "ohim": ohim, "rmf": rmf, "sel": sel}


def run_device(rgb_map, ir_map, rgb_mask, ir_mask, trace=False, **trace_kw):
    """Compile+run the SPMD kernel; returns (per-core results, BassKernelResults)."""
    nc = _get_program()
    in_maps = [_make_in_map(rgb_map, ir_map, rgb_mask, ir_mask, n)
               for n in range(N)]
    res = run_bass_kernel_spmd(nc, in_maps, core_ids=list(range(NCORES)),
                               trace=trace, **trace_kw)
    return res.results, res


def finalize(results, rgb_mask, ir_mask):
    """Host-side -log / masked mean over the per-core row/col sums."""
    total = 0.0
    count = 0.0
    for n in range(N):
        rm = np.asarray(rgb_mask[n]).reshape(HW)
        im = np.asarray(ir_mask[n]).reshape(HW)
        rows = results[n]["rows"].astype(np.float64)
        cols = results[n]["cols"].astype(np.float64)
        row_lm = rows[:, :NK].T.reshape(HW)
        row_lg = rows[:, NK:].T.reshape(HW)
        col_lm = cols[0]
        col_lg = cols[1]
        row = row_lm / (row_lg + EPS)
        col = col_lm / (col_lg + EPS)
        for vec, mask in ((row, rm), (col, im)):
            v = vec * (mask > 0)
            nz = v != 0
            total += -np.log(v[nz]).sum()
            count += nz.sum()
    return np.float32(total / count)


def kernel(rgb_map, ir_map, rgb_mask, ir_mask):
    rgb_map = np.asarray(rgb_map, dtype=np.float32)
    ir_map = np.asarray(ir_map, dtype=np.float32)
    rgb_mask = np.asarray(rgb_mask, dtype=np.int32)
    ir_mask = np.asarray(ir_mask, dtype=np.int32)
    results, _ = run_device(rgb_map, ir_map, rgb_mask, ir_mask)
    return finalize(results, rgb_mask, ir_mask)


# revision 2
# speedup vs baseline: 1.7433x; 1.7433x over previous
"""Trainium2 Bass kernel for nn_CrossPixContrastiveL2.

Per sample (one per NeuronCore, N=8 samples / 8 cores):
  dist[p,q] = ||r_p||^2 + ||i_q||^2 - 2 r_p.i_q          (HW x HW, C=128)
  logit = exp(exp(-dist)/10)
  row[p] = sum_q logit*mask / (sum_q logit + eps)
  col[q] = sum_p logit*mask / (sum_p logit + eps)
  loss = masked mean of -log over foreground/nonzero entries

Device strategy per core:
  - bf16 Gram matmuls (K=C=128) into PSUM. A K=2 "broadcast" matmul first
    seeds PSUM with -||i_q||^2/2 (hi/lo bf16 split for f32-level accuracy),
    so PSUM = r.i - ||i||^2/2. The -||r_p||^2 term enters as the per-
    partition f32 bias of the first ACT pass.
  - ACT pass 1: e1 = Exp(2*PSUM - ||r||^2)   -> exp(-dist), bf16
  - ACT pass 2: logit = Exp(e1/10), fused accum_out -> row sums of logit
  - DVE scalar_tensor_tensor: (im_bcast == rm[p]) * logit with fused
    accum_out -> masked row sums (single op: mask+mul+reduce)
  - Column sums via label-onehot matmuls: lhsT = [onehot(rm) | ones]
    (128 x 22) against logit, accumulated over row tiles -> per-label
    column masses T[l,q]; col_lm[q] = T[im[q],q] via elementwise onehot
    select + K=22 matmul.
Host does the tiny final -log / masked mean over 4x1024 values per core.
"""

import os
from contextlib import ExitStack

import numpy as np
import ml_dtypes

import concourse.bacc as bacc
import concourse.tile as tile
import concourse.mybir as mybir
from concourse.bass_utils import run_bass_kernel_spmd

N, C, H, W = 8, 128, 32, 32
HW = H * W
NCORES = 8
NK = HW // 128          # 8 row tiles of 128 pixels
L = 21                  # label values 0..20
LL = L + 1              # onehot columns + ones column
TEMPERATURE = 10.0
EPS = 1e-6

_BF16 = ml_dtypes.bfloat16

_PROGRAM = None


def _build_program():
    f32 = mybir.dt.float32
    bf16 = mybir.dt.bfloat16
    AF = mybir.ActivationFunctionType
    ALU = mybir.AluOpType

    nc = bacc.Bacc("TRN2", target_bir_lowering=False, debug=False,
                   num_devices=NCORES)

    rgb = nc.dram_tensor("rgb", (C, HW), bf16, kind="ExternalInput").ap()
    irr = nc.dram_tensor("irr", (C, HW), bf16, kind="ExternalInput").ap()
    # hi/lo bf16 split of -||i_q||^2/2 (row0=hi, row1=lo)
    nihb = nc.dram_tensor("nihb", (2, HW), bf16, kind="ExternalInput").ap()
    # two rows of ones (lhsT for the K=2 broadcast matmul)
    ones2 = nc.dram_tensor("ones2", (2, 128), bf16, kind="ExternalInput").ap()
    # -||r_p||^2 in transposed layout [p, k] (ACT bias, f32 exact)
    nrT = nc.dram_tensor("nrT", (128, NK), f32, kind="ExternalInput").ap()
    # ir labels broadcast across partitions (bf16, exact for ints 0..20)
    imb = nc.dram_tensor("imb", (128, HW), bf16, kind="ExternalInput").ap()
    # [p, LL*k + l] = (rm[128k+p] == l) for l<21 ; 1.0 at l=21
    oh = nc.dram_tensor("oh", (128, NK * LL), bf16, kind="ExternalInput").ap()
    # rows 0..20 = onehot of im along q; row 21 = ones
    ohim = nc.dram_tensor("ohim", (LL, HW), f32, kind="ExternalInput").ap()
    # rm labels, transposed layout: [p, k] = rm[128k+p]
    rmf = nc.dram_tensor("rmf", (128, NK), f32, kind="ExternalInput").ap()
    # selector (22,2): col0 = 21 ones then 0 ; col1 = zeros then 1
    sel = nc.dram_tensor("sel", (LL, 2), f32, kind="ExternalInput").ap()

    # outputs: rows[:, 0:NK] = masked row sums, rows[:, NK:2NK] = row sums
    rows = nc.dram_tensor("rows", (128, 2 * NK), f32,
                          kind="ExternalOutput").ap()
    # cols[0] = masked col sums, cols[1] = col sums
    cols = nc.dram_tensor("cols", (2, HW), f32, kind="ExternalOutput").ap()

    with tile.TileContext(nc) as tc, ExitStack() as ctx:
        sb = ctx.enter_context(tc.tile_pool(name="sb", bufs=1))
        work = ctx.enter_context(tc.tile_pool(name="work", bufs=3))
        ps = ctx.enter_context(tc.tile_pool(name="ps", bufs=2, space="PSUM"))
        acc = ctx.enter_context(tc.tile_pool(name="acc", bufs=1, space="PSUM"))
        cps_pool = ctx.enter_context(
            tc.tile_pool(name="cps", bufs=1, space="PSUM"))

        rgb_s = sb.tile([C, HW], bf16)
        nc.sync.dma_start(rgb_s[:], rgb)
        ir_s = sb.tile([C, HW], bf16)
        nc.sync.dma_start(ir_s[:], irr)
        nihb_s = sb.tile([2, HW], bf16)
        nc.sync.dma_start(nihb_s[:], nihb)
        ones2_s = sb.tile([2, 128], bf16)
        nc.sync.dma_start(ones2_s[:], ones2)
        nrT_s = sb.tile([128, NK], f32)
        nc.sync.dma_start(nrT_s[:], nrT)
        imb_s = sb.tile([128, HW], bf16)
        nc.sync.dma_start(imb_s[:], imb)
        oh_s = sb.tile([128, NK * LL], bf16)
        nc.sync.dma_start(oh_s[:], oh)
        ohim_s = sb.tile([LL, HW], f32)
        nc.sync.dma_start(ohim_s[:], ohim)
        rmf_s = sb.tile([128, NK], f32)
        nc.sync.dma_start(rmf_s[:], rmf)
        sel_s = sb.tile([LL, 2], f32)
        nc.sync.dma_start(sel_s[:], sel)

        rows_s = sb.tile([128, 2 * NK], f32)
        # per-label column masses, accumulated across the NK row tiles
        TT = acc.tile([LL, HW], f32)

        for k in range(NK):
            G = ps.tile([128, HW], f32)
            for qh in range(2):
                q = qh * 512
                # PSUM <- -||i||^2/2 broadcast (K=2 hi/lo), then += r.i
                nc.tensor.matmul(G[:, q:q + 512],
                                 ones2_s[:],
                                 nihb_s[:, q:q + 512],
                                 start=True, stop=False)
                nc.tensor.matmul(G[:, q:q + 512],
                                 rgb_s[:, k * 128:(k + 1) * 128],
                                 ir_s[:, q:q + 512],
                                 start=False, stop=True)
            e1 = work.tile([128, HW], bf16, tag="e1")
            nc.scalar.activation(e1[:], G[:], AF.Exp, scale=2.0,
                                 bias=nrT_s[:, k:k + 1])
            logit = work.tile([128, HW], bf16, tag="logit")
            nc.scalar.activation(logit[:], e1[:], AF.Exp,
                                 scale=1.0 / TEMPERATURE,
                                 accum_out=rows_s[:, NK + k:NK + k + 1])
            lm = work.tile([128, HW], bf16, tag="lm")
            nc.vector.scalar_tensor_tensor(
                lm[:], imb_s[:], rmf_s[:, k:k + 1], logit[:],
                op0=ALU.is_equal, op1=ALU.mult,
                accum_out=rows_s[:, k:k + 1])
            for qh in range(2):
                q = qh * 512
                nc.tensor.matmul(TT[:, q:q + 512],
                                 oh_s[:, LL * k:LL * (k + 1)],
                                 logit[:, q:q + 512],
                                 start=(k == 0), stop=(k == NK - 1))

        # col_lm[q] = TT[im[q], q]; col_logit[q] = TT[21, q]
        TM = sb.tile([LL, HW], f32)
        nc.vector.tensor_tensor(TM[:], TT[:], ohim_s[:], op=ALU.mult)
        cps = cps_pool.tile([2, HW], f32)
        for qh in range(2):
            q = qh * 512
            nc.tensor.matmul(cps[:, q:q + 512], sel_s[:], TM[:, q:q + 512],
                             start=True, stop=True)
        cols_s = sb.tile([2, HW], f32)
        nc.vector.tensor_copy(cols_s[:], cps[:])

        nc.sync.dma_start(rows, rows_s[:])
        nc.sync.dma_start(cols, cols_s[:])

    nc.compile()
    return nc


def _get_program():
    global _PROGRAM
    if _PROGRAM is None:
        _PROGRAM = _build_program()
    return _PROGRAM


def _make_in_map(rgb_map, ir_map, rgb_mask, ir_mask, n):
    f32 = np.float32
    rgb32 = np.ascontiguousarray(rgb_map[n].reshape(C, HW), dtype=f32)
    irr32 = np.ascontiguousarray(ir_map[n].reshape(C, HW), dtype=f32)
    rm = rgb_mask[n].reshape(HW)
    im = ir_mask[n].reshape(HW)

    nr = (rgb32 * rgb32).sum(axis=0, dtype=f32)
    ni = (irr32 * irr32).sum(axis=0, dtype=f32)

    x = (-0.5 * ni).astype(f32)
    hi = x.astype(_BF16)
    lo = (x - hi.astype(f32)).astype(_BF16)
    nihb = np.stack([hi, lo])

    ones2 = np.ones((2, 128), dtype=_BF16)

    rmT = rm.reshape(NK, 128).T  # [p, k]
    nrT = np.ascontiguousarray(-nr.reshape(NK, 128).T, dtype=f32)

    imb = np.broadcast_to(im.astype(_BF16), (128, HW)).copy()

    oh = np.zeros((128, NK, LL), dtype=_BF16)
    oh[:, :, :L] = (rmT[:, :, None] == np.arange(L)[None, None, :])
    oh[:, :, L] = 1
    oh = oh.reshape(128, NK * LL)

    ohim = np.zeros((LL, HW), dtype=f32)
    ohim[:L] = (np.arange(L)[:, None] == im[None, :])
    ohim[L] = 1.0

    rmf = np.ascontiguousarray(rmT, dtype=f32)

    sel = np.zeros((LL, 2), dtype=f32)
    sel[:L, 0] = 1.0
    sel[L, 1] = 1.0

    return {"rgb": rgb32.astype(_BF16), "irr": irr32.astype(_BF16),
            "nihb": nihb, "ones2": ones2, "nrT": nrT, "imb": imb,
            "oh": oh, "ohim": ohim, "rmf": rmf, "sel": sel}


def run_device(rgb_map, ir_map, rgb_mask, ir_mask, trace=False, **trace_kw):
    """Compile+run the SPMD kernel; returns (per-core results, BassKernelResults)."""
    nc = _get_program()
    in_maps = [_make_in_map(rgb_map, ir_map, rgb_mask, ir_mask, n)
               for n in range(N)]
    res = run_bass_kernel_spmd(nc, in_maps, core_ids=list(range(NCORES)),
                               trace=trace, **trace_kw)
    return res.results, res


def finalize(results, rgb_mask, ir_mask):
    """Host-side -log / masked mean over the per-core row/col sums."""
    total = 0.0
    count = 0.0
    for n in range(N):
        rm = np.asarray(rgb_mask[n]).reshape(HW)
        im = np.asarray(ir_mask[n]).reshape(HW)
        rows = results[n]["rows"].astype(np.float64)
        cols = results[n]["cols"].astype(np.float64)
        row_lm = rows[:, :NK].T.reshape(HW)
        row_lg = rows[:, NK:].T.reshape(HW)
        col_lm = cols[0]
        col_lg = cols[1]
        row = row_lm / (row_lg + EPS)
        col = col_lm / (col_lg + EPS)
        for vec, mask in ((row, rm), (col, im)):
            v = vec * (mask > 0)
            nz = v != 0
            total += -np.log(v[nz]).sum()
            count += nz.sum()
    return np.float32(total / count)


def kernel(rgb_map, ir_map, rgb_mask, ir_mask):
    rgb_map = np.asarray(rgb_map, dtype=np.float32)
    ir_map = np.asarray(ir_map, dtype=np.float32)
    rgb_mask = np.asarray(rgb_mask, dtype=np.int32)
    ir_mask = np.asarray(ir_mask, dtype=np.int32)
    results, _ = run_device(rgb_map, ir_map, rgb_mask, ir_mask)
    return finalize(results, rgb_mask, ir_mask)
